# revision 13
# baseline (speedup 1.0000x reference)
"""Contrastive loss (NT-Xent style) Trainium2 kernel, symmetric-halved, fp8.

loss = mean_i(log(sum_{j!=i} exp(sim_ij)) - pos_i),  sim = zn @ zn.T / beta,
pos_i = sim[i, (i+N) mod 2N],  zn = z / max(||z||, eps),  z = [x1; x2].

Design vs the bf16 baseline (63.4us):
- Similarity matmuls run in fp8e4 with MatmulPerfMode.DoubleRow (K=256 in one
  shot over the two k-planes) at 0.5 cycles/row -- ~4x less PE time, which
  buys back the scheduling slack everywhere else.
- Normalization is fused into the transposes: instead of scaling z and then
  transposing with an identity, each 128x128 transpose is a *plain* matmul
  z_half^T @ diag(rinv) (same PE cost), so there is no separate scale pass.
- The exp stream (33 tiles x 8 bands of 128x128) is split across all three
  elementwise engines:
    'A'  : Act exp (psum -> bf16 exp tile, accum_out = row sums)
    'R5' : DVE pass1 = tensor_scalar psum*(A/beta)+B -> int16 whose bits are
           the bf16 Schraudolph approximation of exp, then pass2 (4x mode)
           re-reads the bits as bf16 for the row-sum accumulator
    'R3' : Act copies psum -> fp16 (y = sim/beta), DVE does the Schraudolph
           int16 step at 4x, DVE pass2 row-sums
    'R7' : like R3 but Pool (gpsimd) does the pass2 row-sum from SBUF
  i16 = rint(184.6646*y + 16248.5) bitcast bf16 ~= exp(y) (max rel err ~4%,
  near-zero mean; den averages ~8k terms so the loss error stays ~4e-4).
- Row norms (nsq) are square-accumulates split DVE/Act/Pool; rsqrt is the
  Quake bit-trick + 1 Newton step on DVE (no extra act tables).
- Groups per band are [6, 9, 9, 9] tiles so the first exp only needs z tiles
  0..5 (one DMA chunk) -- the Act stream starts ~9us earlier than with
  uniform 11-tile groups.
- Host sends z pre-arranged [128, 40*256] bf16 (partition-major), so each DMA
  chunk is one descriptor per partition; outputs are packed into one tensor.

Sharding: 8 cores x 8 bands. Core c receives z rotated by -1024c rows so the
SPMD program is identical everywhere; the host un-rotates the partial den/pos
outputs, all-reduces them, and applies the final log/mean.
"""

import numpy as np
import ml_dtypes
from contextlib import ExitStack

import concourse.bass as bass
import concourse.tile as tile
from concourse import bacc, mybir
from concourse.bass_utils import run_bass_kernel_spmd

BETA = 0.08
EPS = 1e-8
TWO_N = 8192
D = 256
N_CORES = 8
RPC = TWO_N // N_CORES          # 1024 rows per core
BANDS = RPC // 128              # 8 row bands per core
ZT = 40                         # z row-tiles touched per core (J <= 39)
SW = 33                         # swath width in tiles (d = 0..32)
NA = 32                         # znt tile A holds col tiles 0..31
RSQRT_MAGIC = 0x5F3759DF

GSZ = [6, 9, 9, 9]              # group sizes (tiles); sum = SW
GT0 = [0, 6, 15, 24]            # group start offsets
NG = 4
# colsum d-ranges per group (d = 0 masked diag, d = 32 row-sums only)
CSRANGE = [(1, 6), (6, 15), (15, 24), (24, 32)]

SCHRA_A = 184.6646              # 2^7 / ln 2
SCHRA_B = 16248.5               # 127*2^7 - 7.5 (calibrated, RNE convert)

F32 = mybir.dt.float32
F16 = mybir.dt.float16
I16 = mybir.dt.int16
I32 = mybir.dt.int32
BF16 = mybir.dt.bfloat16
FP8 = mybir.dt.float8e4
AF = mybir.ActivationFunctionType
ALU = mybir.AluOpType
PM = mybir.MatmulPerfMode

# ---- engine split knobs -------------------------------------------------
# exp route per (band, group): 'A', 'R5', 'R3', 'R7'
ROUTE = [
    ['A', 'A', 'A', 'A'],
    ['A', 'R5', 'A', 'A'],
    ['A', 'A', 'R5', 'A'],
    ['A', 'A', 'A', 'R5'],
    ['A', 'R5', 'A', 'A'],
    ['A', 'A', 'R5', 'A'],
    ['A', 'A', 'A', 'R5'],
    ['A', 'R5', 'A', 'A'],
]
# nsq engine per tile index 0..39: V=DVE, A=Act, P=Pool
NSQ_ENG = (['V'] * 5 + ['A'] * 3) + (['V'] * 2 + ['A'] * 2 + ['P'] * 4) + \
          ['P'] * 24
# znt copy engine per 512-col copy (20 of them): V=DVE, A=Act
# first 8 copies (octs 0-1) land in the ramp where Act is idle
COPY_ENG = ['A'] * 8 + ['V'] * 12
# D-matrix build engine per tile: V=DVE only (Pool can't take AP scalars)
D_ENG = ['V'] * 40

TRACE = False
LAST_EXEC_NS = None
LAST_RESULTS = None

_cached_nc = None


def _build():
    nc = bacc.Bacc(
        "TRN2", target_bir_lowering=False, debug=False, num_devices=N_CORES
    )
    z = nc.dram_tensor("z", [128, ZT * D], BF16, kind="ExternalInput").ap()
    cst_f = nc.dram_tensor("cst_f", [128, 256], F32, kind="ExternalInput").ap()
    cst_b = nc.dram_tensor("cst_b", [128, 132], BF16, kind="ExternalInput").ap()
    outp = nc.dram_tensor("outp", [128, 288], F32, kind="ExternalOutput").ap()

    with tile.TileContext(nc) as tc, ExitStack() as ctx:
        const_pool = ctx.enter_context(tc.tile_pool(name="const", bufs=1))
        small = ctx.enter_context(tc.tile_pool(name="small", bufs=1))
        zrow_pool = ctx.enter_context(tc.tile_pool(name="zrow", bufs=1))
        znt_pool = ctx.enter_context(tc.tile_pool(name="znt", bufs=1))
        dmat_pool = ctx.enter_context(tc.tile_pool(name="dmat", bufs=1))
        exp_pool = ctx.enter_context(tc.tile_pool(name="exp", bufs=8))
        scr = ctx.enter_context(tc.tile_pool(name="scr", bufs=4))
        y16_pool = ctx.enter_context(tc.tile_pool(name="y16", bufs=2))
        dump_pool = ctx.enter_context(tc.tile_pool(name="dump", bufs=2))
        tp_psum = ctx.enter_context(tc.tile_pool(name="tp", bufs=1, space="PSUM"))
        mm_psum = ctx.enter_context(tc.tile_pool(name="mm", bufs=2, space="PSUM"))
        cp_psum = ctx.enter_context(tc.tile_pool(name="cp", bufs=1, space="PSUM"))

        # tiny activation first so the Exp/Square table set loads at t~0
        warm = small.tile([128, 1], F32, tag="warm")
        nc.vector.memset(warm[:], 0.0)
        wdump = small.tile([128, 1], BF16, tag="wdump")
        nc.scalar.activation(wdump[:], warm[:], AF.Exp)

        # input DMAs: z chunks first (oct0 smallest-latency), consts between
        zrow = zrow_pool.tile([128, ZT * D], BF16, tag="zrow", name="zrow")
        nc.sync.dma_start(zrow[:, 0 : 8 * D], z[:, 0 : 8 * D])
        cstf_sb = const_pool.tile([128, 256], F32, tag="cstf")
        nc.sync.dma_start(cstf_sb[:], cst_f[:, :])
        cstb_sb = const_pool.tile([128, 132], BF16, tag="cstb")
        nc.sync.dma_start(cstb_sb[:], cst_b[:, :])
        nc.sync.dma_start(zrow[:, 8 * D : 16 * D], z[:, 8 * D : 16 * D])
        nc.sync.dma_start(zrow[:, 16 * D : 28 * D], z[:, 16 * D : 28 * D])
        nc.sync.dma_start(zrow[:, 28 * D : 40 * D], z[:, 28 * D : 40 * D])

        eyef = cstf_sb[:, 0:128]
        negeye4 = cstf_sb[:, 128:256]
        eye_b = cstb_sb[:, 0:128]
        ones_b = cstb_sb[:, 128:129]

        def ztile(i):
            return zrow[:, D * i : D * (i + 1)]

        nsq = small.tile([128, ZT], F32, tag="nsq")
        rinv = small.tile([128, ZT], F32, tag="rinv")
        rscr = small.tile([128, ZT], F32, tag="rscr")
        # per-band layout: [36i:36i+4) denrow, [36i+4:36i+35) colp, [36i+35] pos
        out_sb = small.tile([128, 288], F32, tag="out")

        dmats = dmat_pool.tile([128, ZT * 128], BF16, tag="dm", name="dmats")

        # znt: normalized-transposed z in fp8 k-plane layout
        # A[p, 4096k + 128J + c] = zn[128J + c, 128k + p] for J < 32
        znt_a = znt_pool.tile([128, 2 * 4096], FP8, tag="znta", name="znt_a")
        znt_b = znt_pool.tile([128, 2 * 1024], FP8, tag="zntb", name="znt_b")

        def nsq_tile(i):
            eng = NSQ_ENG[i]
            if eng == 'A':
                dump = scr.tile([128, D], BF16, tag="sqdump")
                nc.scalar.activation(
                    dump[:], ztile(i), AF.Square, accum_out=nsq[:, i : i + 1]
                )
            elif eng == 'V':
                dump = scr.tile([128, D], BF16, tag="sqdump")
                nc.vector.scalar_tensor_tensor(
                    out=dump[:],
                    in0=ztile(i),
                    scalar=0.0,
                    in1=ztile(i),
                    op0=ALU.bypass,
                    op1=ALU.mult,
                    accum_out=nsq[:, i : i + 1],
                )
            else:
                # Pool squares; DVE row-sums the squares at 4x (cheap)
                dump = scr.tile([128, D], BF16, tag="sqdump")
                nc.gpsimd.tensor_tensor(dump[:], ztile(i), ztile(i), ALU.mult)
                dump2 = scr.tile([128, D], BF16, tag="sqdump2")
                nc.vector.tensor_scalar(
                    out=dump2[:], in0=dump[:], scalar1=1.0, scalar2=0.0,
                    op0=ALU.mult, op1=ALU.add,
                    accum_out=nsq[:, i : i + 1],
                )

        def rsqrt_batch(c0, c1):
            # rinv = rsqrt(max(nsq, eps^2)): Quake seed + 1 Newton iteration
            ns, y, t = nsq[:, c0:c1], rinv[:, c0:c1], rscr[:, c0:c1]
            nc.vector.tensor_scalar_max(ns, ns, EPS * EPS)
            nc.vector.tensor_scalar(
                out=y.bitcast(I32), in0=ns.bitcast(I32),
                scalar1=1, scalar2=None, op0=ALU.arith_shift_right,
            )
            nc.vector.tensor_scalar(
                out=y.bitcast(I32), in0=y.bitcast(I32),
                scalar1=-1, scalar2=RSQRT_MAGIC, op0=ALU.mult, op1=ALU.add,
            )
            nc.vector.tensor_tensor(t, y, y, ALU.mult)
            nc.vector.tensor_tensor(t, t, ns, ALU.mult)
            nc.vector.tensor_scalar(
                out=t, in0=t, scalar1=-0.5, scalar2=1.5,
                op0=ALU.mult, op1=ALU.add,
            )
            nc.vector.tensor_tensor(y, y, t, ALU.mult)

        def dmat_build(i):
            # D_i = eye * rinv_i  (diag matrix; off-diag stays 0)
            e = nc.vector if D_ENG[i] == 'V' else nc.gpsimd
            e.tensor_scalar(
                out=dmats[:, 128 * i : 128 * (i + 1)],
                in0=eye_b[:, :],
                scalar1=rinv[:, i : i + 1],
                scalar2=None,
                op0=ALU.mult,
            )

        _copy_n = [0]

        def transpose_quad(q, k):
            # tiles 4q..4q+3, k-plane k: 4 scaled transposes into one psum
            # tile, then one 512-col copy into fp8 znt
            pb = tp_psum.tile([128, 512], F32, tag="tp")
            for e in range(4):
                i = 4 * q + e
                nc.tensor.matmul(
                    pb[:, 128 * e : 128 * (e + 1)],
                    ztile(i)[:, 128 * k : 128 * (k + 1)],
                    dmats[:, 128 * i : 128 * (i + 1)],
                    start=True, stop=True,
                )
            J0 = 4 * q
            if J0 < NA:
                dst = znt_a[:, 4096 * k + 128 * J0 : 4096 * k + 128 * J0 + 512]
            else:
                dst = znt_b[:, 1024 * k + 128 * (J0 - NA) : 1024 * k + 128 * (J0 - NA) + 512]
            eng = COPY_ENG[_copy_n[0] % len(COPY_ENG)]
            _copy_n[0] += 1
            if eng == 'A':
                nc.scalar.copy(dst, pb[:])
            else:
                nc.vector.tensor_copy(dst, pb[:])

        def prologue(t0, t1):
            for i in range(t0, t1):
                nsq_tile(i)
            rsqrt_batch(t0, t1)
            for i in range(t0, t1):
                dmat_build(i)
            for q in range(t0 // 4, t1 // 4):
                for k in range(2):
                    transpose_quad(q, k)

        def znt_dr(J, w_tiles, k2):
            # DoubleRow AP [128, 2, 128*w_tiles] starting at col tile J
            if J < NA:
                base = znt_a[:, :].rearrange("p (k c) -> p k c", k=2)
                return base[:, :, 128 * J : 128 * (J + w_tiles)]
            base = znt_b[:, :].rearrange("p (k c) -> p k c", k=2)
            return base[:, :, 128 * (J - NA) : 128 * (J - NA + w_tiles)]

        exp_ts = [
            exp_pool.tile([128, SW * 128], BF16, tag="exp", name=f"exp{i}")
            for i in range(BANDS)
        ]
        colp = cp_psum.tile([128, 31 * BANDS], F32, tag="cp")

        def do_group(i, g):
            t0, nb = GT0[g], GSZ[g]
            gw = nb * 128
            exp_t = exp_ts[i]
            exp_sl = exp_t[:, 128 * t0 : 128 * t0 + gw]
            pg = mm_psum.tile([128, 1152], F32, tag="mm")
            # fp8 DoubleRow matmuls, <=2 tiles (256 cols) per instruction
            b = 0
            while b < nb:
                J = i + t0 + b
                seg = (NA - J) if J < NA else (ZT - J)
                w = min(2, nb - b, seg)
                nc.tensor.matmul(
                    pg[:, 128 * b : 128 * b + 128 * w],
                    znt_dr(i, 1, 2),
                    znt_dr(J, w, 2),
                    start=True, stop=True,
                    perf_mode=PM.DoubleRow,
                )
                b += w
            if g == 0:
                # mask self-similarity: diag -> sim-4 => y=-37.5, exp ~ 5e-17
                nc.vector.tensor_add(pg[:, 0:128], pg[:, 0:128], negeye4)
            if g == NG - 1:
                # d = 32 (last 128 cols): positive-pair block; diag = pos
                pdump = scr.tile([128, 128], F32, tag="pdump")
                nc.vector.scalar_tensor_tensor(
                    out=pdump[:],
                    in0=pg[:, gw - 128 : gw],
                    scalar=1.0 / BETA,
                    in1=eyef,
                    op0=ALU.mult,
                    op1=ALU.mult,
                    accum_out=out_sb[:, 36 * i + 35 : 36 * i + 36],
                )
            den_col = out_sb[:, 36 * i + g : 36 * i + g + 1]
            route = ROUTE[i][g]
            if route == 'A':
                nc.scalar.activation(
                    exp_sl, pg[:, 0:gw], AF.Exp,
                    scale=1.0 / BETA, accum_out=den_col,
                )
            elif route == 'R5':
                nc.vector.tensor_scalar(
                    out=exp_sl.bitcast(I16), in0=pg[:, 0:gw],
                    scalar1=SCHRA_A / BETA, scalar2=SCHRA_B,
                    op0=ALU.mult, op1=ALU.add,
                )
                dump = dump_pool.tile([128, 1152], BF16, tag="dump")
                nc.vector.tensor_scalar(
                    out=dump[:, 0:gw], in0=exp_sl,
                    scalar1=1.0, scalar2=0.0, op0=ALU.mult, op1=ALU.add,
                    accum_out=den_col,
                )
            else:  # R3: Act stages y=sim/beta in fp16, DVE does both 4x passes
                y16 = y16_pool.tile([128, 1152], F16, tag="y16")
                nc.scalar.activation(
                    y16[:, 0:gw], pg[:, 0:gw], AF.Copy, scale=1.0 / BETA
                )
                nc.vector.tensor_scalar(
                    out=exp_sl.bitcast(I16), in0=y16[:, 0:gw],
                    scalar1=SCHRA_A, scalar2=SCHRA_B,
                    op0=ALU.mult, op1=ALU.add,
                )
                dump = dump_pool.tile([128, 1152], BF16, tag="dump")
                nc.vector.tensor_scalar(
                    out=dump[:, 0:gw], in0=exp_sl,
                    scalar1=1.0, scalar2=0.0, op0=ALU.mult, op1=ALU.add,
                    accum_out=den_col,
                )
            # column sums via ones-matmuls (free on PE)
            d0, d1 = CSRANGE[g]
            for d in range(d0, d1):
                nc.tensor.matmul(
                    colp[:, 31 * i + d - 1 : 31 * i + d],
                    exp_t[:, 128 * d : 128 * (d + 1)],
                    ones_b,
                    start=True, stop=True,
                )
            if g == NG - 1:
                # stage this band's column sums; host does the final gather
                nc.vector.tensor_copy(
                    out_sb[:, 36 * i + 4 : 36 * i + 35],
                    colp[:, 31 * i : 31 * (i + 1)],
                )

        # ---------------- schedule ----------------
        prologue(0, 8)
        for i in range(3):
            do_group(i, 0)
        prologue(8, 16)
        for i in range(3, BANDS):
            do_group(i, 0)
        prologue(16, 24)
        for i in range(BANDS):
            do_group(i, 1)
        prologue(24, 32)
        for i in range(BANDS):
            do_group(i, 2)
        prologue(32, 40)
        for i in range(BANDS):
            do_group(i, 3)

        nc.sync.dma_start(outp[:, :], out_sb[:])

    nc.compile()
    return nc


def _get_nc():
    global _cached_nc
    if _cached_nc is None:
        _cached_nc = _build()
    return _cached_nc


def kernel(x1: np.ndarray, x2: np.ndarray) -> np.ndarray:
    global LAST_EXEC_NS, LAST_RESULTS
    z = np.concatenate(
        [np.asarray(x1, dtype=np.float32), np.asarray(x2, dtype=np.float32)], axis=0
    )
    eye = np.eye(128, dtype=np.float32)
    cst_f = np.concatenate([eye, -4.0 * eye], axis=1)
    cst_b = np.concatenate(
        [eye, np.ones((128, 4), dtype=np.float32)], axis=1
    ).astype(ml_dtypes.bfloat16)
    in_maps = []
    for c in range(N_CORES):
        zc = np.roll(z, -RPC * c, axis=0)[: ZT * 128].astype(ml_dtypes.bfloat16)
        # [40*128, 256] -> [128, 40*256] partition-major
        zc = np.ascontiguousarray(
            zc.reshape(ZT, 128, D).transpose(1, 0, 2).reshape(128, ZT * D)
        )
        in_maps.append({"z": zc, "cst_f": cst_f, "cst_b": cst_b})
    nc = _get_nc()
    res = run_bass_kernel_spmd(nc, in_maps, list(range(N_CORES)), trace=TRACE)
    LAST_EXEC_NS = res.exec_time_ns
    LAST_RESULTS = res

    # gather: un-rotate and all-reduce den/pos, then log + mean on host
    den = np.zeros(TWO_N, dtype=np.float64)
    pos = np.zeros(TWO_N, dtype=np.float64)
    for c, r in enumerate(res.results):
        out = r["outp"].astype(np.float64)   # [128, 288]
        off = RPC * c
        p = np.arange(128)
        for i in range(BANDS):
            rows = (128 * i + p + off) % TWO_N
            den[rows] += out[:, 36 * i : 36 * i + 4].sum(axis=1)
            pos[rows] = out[:, 36 * i + 35]
            for d in range(1, 32):
                jrows = (128 * (i + d) + p + off) % TWO_N
                den[jrows] += out[:, 36 * i + 4 + d - 1]
    loss = np.mean(np.log(den) - pos)
    return np.array(loss, dtype=np.float32)


# revision 16
# speedup vs baseline: 1.1046x; 1.1046x over previous
"""Contrastive loss (NT-Xent style) Trainium2 kernel, symmetric-halved, fp8.

loss = mean_i(log(sum_{j!=i} exp(sim_ij)) - pos_i),  sim = zn @ zn.T / beta,
pos_i = sim[i, (i+N) mod 2N],  zn = z / max(||z||, eps),  z = [x1; x2].

Design vs the bf16 baseline (63.4us):
- Similarity matmuls run in fp8e4 with MatmulPerfMode.DoubleRow (K=256 in one
  shot over the two k-planes) at 0.5 cycles/row -- ~4x less PE time, which
  buys back the scheduling slack everywhere else.
- Normalization is fused into the transposes: instead of scaling z and then
  transposing with an identity, each 128x128 transpose is a *plain* matmul
  z_half^T @ diag(rinv) (same PE cost), so there is no separate scale pass.
- The exp stream (33 tiles x 8 bands of 128x128) is split across all three
  elementwise engines:
    'A'  : Act exp (psum -> bf16 exp tile, accum_out = row sums)
    'R5' : DVE pass1 = tensor_scalar psum*(A/beta)+B -> int16 whose bits are
           the bf16 Schraudolph approximation of exp, then pass2 (4x mode)
           re-reads the bits as bf16 for the row-sum accumulator
    'R3' : Act copies psum -> fp16 (y = sim/beta), DVE does the Schraudolph
           int16 step at 4x, DVE pass2 row-sums
    'R7' : like R3 but Pool (gpsimd) does the pass2 row-sum from SBUF
  i16 = rint(184.6646*y + 16248.5) bitcast bf16 ~= exp(y) (max rel err ~4%,
  near-zero mean; den averages ~8k terms so the loss error stays ~4e-4).
- Row norms (nsq) are square-accumulates split DVE/Act/Pool; rsqrt is the
  Quake bit-trick + 1 Newton step on DVE (no extra act tables).
- Groups per band are [6, 9, 9, 9] tiles so the first exp only needs z tiles
  0..5 (one DMA chunk) -- the Act stream starts ~9us earlier than with
  uniform 11-tile groups.
- Host sends z pre-arranged [128, 40*256] bf16 (partition-major), so each DMA
  chunk is one descriptor per partition; outputs are packed into one tensor.

Sharding: 8 cores x 8 bands. Core c receives z rotated by -1024c rows so the
SPMD program is identical everywhere; the host un-rotates the partial den/pos
outputs, all-reduces them, and applies the final log/mean.
"""

import numpy as np
import ml_dtypes
from contextlib import ExitStack

import concourse.bass as bass
import concourse.tile as tile
from concourse import bacc, mybir
from concourse.bass_utils import run_bass_kernel_spmd

BETA = 0.08
EPS = 1e-8
TWO_N = 8192
D = 256
N_CORES = 8
RPC = TWO_N // N_CORES          # 1024 rows per core
BANDS = RPC // 128              # 8 row bands per core
ZT = 40                         # z row-tiles touched per core (J <= 39)
SW = 33                         # swath width in tiles (d = 0..32)
NA = 32                         # znt tile A holds col tiles 0..31
RSQRT_MAGIC = 0x5F3759DF

GSZ = [6, 9, 9, 9]              # group sizes (tiles); sum = SW
GT0 = [0, 6, 15, 24]            # group start offsets
NG = 4
# colsum d-ranges per group (d = 0 masked diag, d = 32 row-sums only)
CSRANGE = [(1, 6), (6, 15), (15, 24), (24, 32)]

SCHRA_A = 184.6646              # 2^7 / ln 2
SCHRA_B = 16248.5               # 127*2^7 - 7.5 (calibrated, RNE convert)

F32 = mybir.dt.float32
F16 = mybir.dt.float16
I16 = mybir.dt.int16
I32 = mybir.dt.int32
BF16 = mybir.dt.bfloat16
FP8 = mybir.dt.float8e4
AF = mybir.ActivationFunctionType
ALU = mybir.AluOpType
PM = mybir.MatmulPerfMode

# ---- engine split knobs -------------------------------------------------
# exp route per (band, group): 'A', 'R5', 'R3', 'R7'
ROUTE = [
    ['A', 'A', 'A', 'A'],
    ['A', 'R5', 'A', 'A'],
    ['A', 'A', 'R5', 'A'],
    ['A', 'A', 'A', 'R5'],
    ['A', 'R5', 'A', 'A'],
    ['A', 'A', 'R5', 'A'],
    ['A', 'A', 'A', 'R5'],
    ['A', 'R5', 'A', 'A'],
]
# nsq engine per tile index 0..39: V=DVE, A=Act, P=Pool.
# Act only during the ramp (octs 0-1); Pool load spread so its slow serial
# squares never gate a soon-needed oct.
NSQ_ENG = (['V'] * 5 + ['A'] * 3) + (['V'] * 2 + ['A'] * 2 + ['P'] * 4) + \
          (['V'] * 4 + ['P'] * 4) + (['V'] * 3 + ['P'] * 5) + \
          (['V'] * 2 + ['P'] * 6)
# znt copy engine per 512-col copy (20 of them): V=DVE, A=Act
# first 8 copies (octs 0-1) land in the ramp where Act is idle
COPY_ENG = ['A'] * 8 + ['V'] * 12
# D-matrix build engine per tile: V=DVE only (Pool can't take AP scalars)
D_ENG = ['V'] * 40

TRACE = False
LAST_EXEC_NS = None
LAST_RESULTS = None

_cached_nc = None


def _build():
    nc = bacc.Bacc(
        "TRN2", target_bir_lowering=False, debug=False, num_devices=N_CORES
    )
    z = nc.dram_tensor("z", [128, ZT * D], BF16, kind="ExternalInput").ap()
    cst_f = nc.dram_tensor("cst_f", [128, 256], F32, kind="ExternalInput").ap()
    cst_b = nc.dram_tensor("cst_b", [128, 132], BF16, kind="ExternalInput").ap()
    outp = nc.dram_tensor("outp", [128, 288], F32, kind="ExternalOutput").ap()

    with tile.TileContext(nc) as tc, ExitStack() as ctx:
        const_pool = ctx.enter_context(tc.tile_pool(name="const", bufs=1))
        small = ctx.enter_context(tc.tile_pool(name="small", bufs=1))
        zrow_pool = ctx.enter_context(tc.tile_pool(name="zrow", bufs=1))
        znt_pool = ctx.enter_context(tc.tile_pool(name="znt", bufs=1))
        dmat_pool = ctx.enter_context(tc.tile_pool(name="dmat", bufs=1))
        exp_pool = ctx.enter_context(tc.tile_pool(name="exp", bufs=8))
        scr = ctx.enter_context(tc.tile_pool(name="scr", bufs=4))
        y16_pool = ctx.enter_context(tc.tile_pool(name="y16", bufs=2))
        dump_pool = ctx.enter_context(tc.tile_pool(name="dump", bufs=2))
        tp_psum = ctx.enter_context(tc.tile_pool(name="tp", bufs=1, space="PSUM"))
        mm_psum = ctx.enter_context(tc.tile_pool(name="mm", bufs=2, space="PSUM"))
        cp_psum = ctx.enter_context(tc.tile_pool(name="cp", bufs=1, space="PSUM"))

        # tiny activation first so the Exp/Square table set loads at t~0
        warm = small.tile([128, 1], F32, tag="warm")
        nc.vector.memset(warm[:], 0.0)
        wdump = small.tile([128, 1], BF16, tag="wdump")
        nc.scalar.activation(wdump[:], warm[:], AF.Exp)

        # input DMAs: z chunks first (oct0 smallest-latency), consts between
        zrow = zrow_pool.tile([128, ZT * D], BF16, tag="zrow", name="zrow")
        nc.sync.dma_start(zrow[:, 0 : 8 * D], z[:, 0 : 8 * D])
        cstf_sb = const_pool.tile([128, 256], F32, tag="cstf")
        nc.sync.dma_start(cstf_sb[:], cst_f[:, :])
        cstb_sb = const_pool.tile([128, 132], BF16, tag="cstb")
        nc.sync.dma_start(cstb_sb[:], cst_b[:, :])
        nc.sync.dma_start(zrow[:, 8 * D : 16 * D], z[:, 8 * D : 16 * D])
        nc.sync.dma_start(zrow[:, 16 * D : 28 * D], z[:, 16 * D : 28 * D])
        nc.sync.dma_start(zrow[:, 28 * D : 40 * D], z[:, 28 * D : 40 * D])

        eyef = cstf_sb[:, 0:128]
        negeye4 = cstf_sb[:, 128:256]
        eye_b = cstb_sb[:, 0:128]
        ones_b = cstb_sb[:, 128:129]

        def ztile(i):
            return zrow[:, D * i : D * (i + 1)]

        nsq = small.tile([128, ZT], F32, tag="nsq")
        rinv = small.tile([128, ZT], F32, tag="rinv")
        rscr = small.tile([128, ZT], F32, tag="rscr")
        # per-band layout: [36i:36i+4) denrow, [36i+4:36i+35) colp, [36i+35] pos
        out_sb = small.tile([128, 288], F32, tag="out")

        dmats = dmat_pool.tile([128, ZT * 128], BF16, tag="dm", name="dmats")

        # znt: normalized-transposed z in fp8 k-plane layout
        # A[p, 4096k + 128J + c] = zn[128J + c, 128k + p] for J < 32
        znt_a = znt_pool.tile([128, 2 * 4096], FP8, tag="znta", name="znt_a")
        znt_b = znt_pool.tile([128, 2 * 1024], FP8, tag="zntb", name="znt_b")

        def nsq_tile(i):
            eng = NSQ_ENG[i]
            if eng == 'A':
                dump = scr.tile([128, D], BF16, tag="sqdump")
                nc.scalar.activation(
                    dump[:], ztile(i), AF.Square, accum_out=nsq[:, i : i + 1]
                )
            elif eng == 'V':
                dump = scr.tile([128, D], BF16, tag="sqdump")
                nc.vector.scalar_tensor_tensor(
                    out=dump[:],
                    in0=ztile(i),
                    scalar=0.0,
                    in1=ztile(i),
                    op0=ALU.bypass,
                    op1=ALU.mult,
                    accum_out=nsq[:, i : i + 1],
                )
            else:
                # Pool squares; DVE row-sums the squares at 4x (cheap)
                dump = scr.tile([128, D], BF16, tag="sqdump")
                nc.gpsimd.tensor_tensor(dump[:], ztile(i), ztile(i), ALU.mult)
                dump2 = scr.tile([128, D], BF16, tag="sqdump2")
                nc.vector.tensor_scalar(
                    out=dump2[:], in0=dump[:], scalar1=1.0, scalar2=0.0,
                    op0=ALU.mult, op1=ALU.add,
                    accum_out=nsq[:, i : i + 1],
                )

        def rsqrt_batch(c0, c1):
            # rinv = rsqrt(max(nsq, eps^2)): Quake seed + 1 Newton iteration
            ns, y, t = nsq[:, c0:c1], rinv[:, c0:c1], rscr[:, c0:c1]
            nc.vector.tensor_scalar_max(ns, ns, EPS * EPS)
            nc.vector.tensor_scalar(
                out=y.bitcast(I32), in0=ns.bitcast(I32),
                scalar1=1, scalar2=None, op0=ALU.arith_shift_right,
            )
            nc.vector.tensor_scalar(
                out=y.bitcast(I32), in0=y.bitcast(I32),
                scalar1=-1, scalar2=RSQRT_MAGIC, op0=ALU.mult, op1=ALU.add,
            )
            nc.vector.tensor_tensor(t, y, y, ALU.mult)
            nc.vector.tensor_tensor(t, t, ns, ALU.mult)
            nc.vector.tensor_scalar(
                out=t, in0=t, scalar1=-0.5, scalar2=1.5,
                op0=ALU.mult, op1=ALU.add,
            )
            nc.vector.tensor_tensor(y, y, t, ALU.mult)

        def dmat_build(i):
            # D_i = eye * rinv_i  (diag matrix; off-diag stays 0)
            e = nc.vector if D_ENG[i] == 'V' else nc.gpsimd
            e.tensor_scalar(
                out=dmats[:, 128 * i : 128 * (i + 1)],
                in0=eye_b[:, :],
                scalar1=rinv[:, i : i + 1],
                scalar2=None,
                op0=ALU.mult,
            )

        _copy_n = [0]

        def transpose_quad(q, k):
            # tiles 4q..4q+3, k-plane k: 4 scaled transposes into one psum
            # tile, then one 512-col copy into fp8 znt
            pb = tp_psum.tile([128, 512], F32, tag="tp")
            for e in range(4):
                i = 4 * q + e
                nc.tensor.matmul(
                    pb[:, 128 * e : 128 * (e + 1)],
                    ztile(i)[:, 128 * k : 128 * (k + 1)],
                    dmats[:, 128 * i : 128 * (i + 1)],
                    start=True, stop=True,
                )
            J0 = 4 * q
            if J0 < NA:
                dst = znt_a[:, 4096 * k + 128 * J0 : 4096 * k + 128 * J0 + 512]
            else:
                dst = znt_b[:, 1024 * k + 128 * (J0 - NA) : 1024 * k + 128 * (J0 - NA) + 512]
            eng = COPY_ENG[_copy_n[0] % len(COPY_ENG)]
            _copy_n[0] += 1
            if eng == 'A':
                nc.scalar.copy(dst, pb[:])
            else:
                nc.vector.tensor_copy(dst, pb[:])

        def prologue_norms(t0, t1):
            for i in range(t0, t1):
                nsq_tile(i)
            rsqrt_batch(t0, t1)
            for i in range(t0, t1):
                dmat_build(i)

        def prologue_tp(t0, t1):
            for q in range(t0 // 4, t1 // 4):
                for k in range(2):
                    transpose_quad(q, k)

        def prologue(t0, t1):
            prologue_norms(t0, t1)
            prologue_tp(t0, t1)

        def znt_dr(J, w_tiles, k2):
            # DoubleRow AP [128, 2, 128*w_tiles] starting at col tile J
            if J < NA:
                base = znt_a[:, :].rearrange("p (k c) -> p k c", k=2)
                return base[:, :, 128 * J : 128 * (J + w_tiles)]
            base = znt_b[:, :].rearrange("p (k c) -> p k c", k=2)
            return base[:, :, 128 * (J - NA) : 128 * (J - NA + w_tiles)]

        exp_ts = [
            exp_pool.tile([128, SW * 128], BF16, tag="exp", name=f"exp{i}")
            for i in range(BANDS)
        ]
        colp = cp_psum.tile([128, 31 * BANDS], F32, tag="cp")

        def do_group(i, g):
            t0, nb = GT0[g], GSZ[g]
            gw = nb * 128
            exp_t = exp_ts[i]
            exp_sl = exp_t[:, 128 * t0 : 128 * t0 + gw]
            pg = mm_psum.tile([128, 1152], F32, tag="mm")
            # fp8 DoubleRow matmuls, <=2 tiles (256 cols) per instruction
            b = 0
            while b < nb:
                J = i + t0 + b
                seg = (NA - J) if J < NA else (ZT - J)
                w = min(2, nb - b, seg)
                nc.tensor.matmul(
                    pg[:, 128 * b : 128 * b + 128 * w],
                    znt_dr(i, 1, 2),
                    znt_dr(J, w, 2),
                    start=True, stop=True,
                    perf_mode=PM.DoubleRow,
                )
                b += w
            if g == 0:
                # mask self-similarity: diag -> sim-4 => y=-37.5, exp ~ 5e-17
                nc.vector.tensor_add(pg[:, 0:128], pg[:, 0:128], negeye4)
            if g == NG - 1:
                # d = 32 (last 128 cols): positive-pair block; diag = pos
                pdump = scr.tile([128, 128], F32, tag="pdump")
                nc.vector.scalar_tensor_tensor(
                    out=pdump[:],
                    in0=pg[:, gw - 128 : gw],
                    scalar=1.0 / BETA,
                    in1=eyef,
                    op0=ALU.mult,
                    op1=ALU.mult,
                    accum_out=out_sb[:, 36 * i + 35 : 36 * i + 36],
                )
            den_col = out_sb[:, 36 * i + g : 36 * i + g + 1]
            route = ROUTE[i][g]
            if route == 'A':
                nc.scalar.activation(
                    exp_sl, pg[:, 0:gw], AF.Exp,
                    scale=1.0 / BETA, accum_out=den_col,
                )
            elif route == 'R5':
                nc.vector.tensor_scalar(
                    out=exp_sl.bitcast(I16), in0=pg[:, 0:gw],
                    scalar1=SCHRA_A / BETA, scalar2=SCHRA_B,
                    op0=ALU.mult, op1=ALU.add,
                )
                dump = dump_pool.tile([128, 1152], BF16, tag="dump")
                nc.vector.tensor_scalar(
                    out=dump[:, 0:gw], in0=exp_sl,
                    scalar1=1.0, scalar2=0.0, op0=ALU.mult, op1=ALU.add,
                    accum_out=den_col,
                )
            else:  # R3: Act stages y=sim/beta in fp16, DVE does both 4x passes
                y16 = y16_pool.tile([128, 1152], F16, tag="y16")
                nc.scalar.activation(
                    y16[:, 0:gw], pg[:, 0:gw], AF.Copy, scale=1.0 / BETA
                )
                nc.vector.tensor_scalar(
                    out=exp_sl.bitcast(I16), in0=y16[:, 0:gw],
                    scalar1=SCHRA_A, scalar2=SCHRA_B,
                    op0=ALU.mult, op1=ALU.add,
                )
                dump = dump_pool.tile([128, 1152], BF16, tag="dump")
                nc.vector.tensor_scalar(
                    out=dump[:, 0:gw], in0=exp_sl,
                    scalar1=1.0, scalar2=0.0, op0=ALU.mult, op1=ALU.add,
                    accum_out=den_col,
                )
            # column sums via ones-matmuls (free on PE)
            d0, d1 = CSRANGE[g]
            for d in range(d0, d1):
                nc.tensor.matmul(
                    colp[:, 31 * i + d - 1 : 31 * i + d],
                    exp_t[:, 128 * d : 128 * (d + 1)],
                    ones_b,
                    start=True, stop=True,
                )
            if g == NG - 1:
                # stage this band's column sums; host does the final gather
                nc.vector.tensor_copy(
                    out_sb[:, 36 * i + 4 : 36 * i + 35],
                    colp[:, 31 * i : 31 * (i + 1)],
                )

        # ---------------- schedule ----------------
        # norms are emitted well before the transposes that need them, so the
        # slow Pool squares never gate a soon-to-be-needed oct
        prologue(0, 8)
        for i in range(3):
            do_group(i, 0)
        prologue(8, 16)
        for i in range(3, BANDS):
            do_group(i, 0)
        prologue_norms(16, 24)
        prologue_tp(16, 24)
        prologue_norms(24, 32)
        for i in range(BANDS):
            do_group(i, 1)
        prologue_tp(24, 32)
        prologue_norms(32, 40)
        for i in range(BANDS):
            do_group(i, 2)
        prologue_tp(32, 40)
        for i in range(BANDS):
            do_group(i, 3)

        nc.sync.dma_start(outp[:, :], out_sb[:])

    nc.compile()
    return nc


def _get_nc():
    global _cached_nc
    if _cached_nc is None:
        _cached_nc = _build()
    return _cached_nc


def kernel(x1: np.ndarray, x2: np.ndarray) -> np.ndarray:
    global LAST_EXEC_NS, LAST_RESULTS
    z = np.concatenate(
        [np.asarray(x1, dtype=np.float32), np.asarray(x2, dtype=np.float32)], axis=0
    )
    eye = np.eye(128, dtype=np.float32)
    cst_f = np.concatenate([eye, -4.0 * eye], axis=1)
    cst_b = np.concatenate(
        [eye, np.ones((128, 4), dtype=np.float32)], axis=1
    ).astype(ml_dtypes.bfloat16)
    in_maps = []
    for c in range(N_CORES):
        zc = np.roll(z, -RPC * c, axis=0)[: ZT * 128].astype(ml_dtypes.bfloat16)
        # [40*128, 256] -> [128, 40*256] partition-major
        zc = np.ascontiguousarray(
            zc.reshape(ZT, 128, D).transpose(1, 0, 2).reshape(128, ZT * D)
        )
        in_maps.append({"z": zc, "cst_f": cst_f, "cst_b": cst_b})
    nc = _get_nc()
    res = run_bass_kernel_spmd(nc, in_maps, list(range(N_CORES)), trace=TRACE)
    LAST_EXEC_NS = res.exec_time_ns
    LAST_RESULTS = res

    # gather: un-rotate and all-reduce den/pos, then log + mean on host
    den = np.zeros(TWO_N, dtype=np.float64)
    pos = np.zeros(TWO_N, dtype=np.float64)
    for c, r in enumerate(res.results):
        out = r["outp"].astype(np.float64)   # [128, 288]
        off = RPC * c
        p = np.arange(128)
        for i in range(BANDS):
            rows = (128 * i + p + off) % TWO_N
            den[rows] += out[:, 36 * i : 36 * i + 4].sum(axis=1)
            pos[rows] = out[:, 36 * i + 35]
            for d in range(1, 32):
                jrows = (128 * (i + d) + p + off) % TWO_N
                den[jrows] += out[:, 36 * i + 4 + d - 1]
    loss = np.mean(np.log(den) - pos)
    return np.array(loss, dtype=np.float32)
